# revision 73
# baseline (speedup 1.0000x reference)
"""Trainium2 Bass kernel for nn_NodeModel (GNN message passing + external
attention + MLP), SPMD across 8 NeuronCores.

Sharding: nodes (and their incoming edges) are partitioned by destination-node
range across the 8 cores; small params are replicated. Host pre-sorts edges by
destination 128-node window.

Design (transposed pipeline, bf16 everywhere):
  - segment-sum emits agg TRANSPOSED: aggT[h, n] += e_chunk[e, h]^T @ onehot[e, n]
    (one bf16 matmul per 128-edge chunk; one-hot built by DVE is_equal at 4x).
  - x^T / ub^T are pre-transposed host-side so the attention/MLP matmul needs
    no on-device transposes of the concat features.
  - LayerNorm centering is folded into the weights EXACTLY:
      sum_c (cat_c - mu) W_c = sum_c cat_c (W_c - colmean(W))
    so the device never materializes mu for the matmul; r (=1/std) is applied
    via per-partition activation scales (softmax exp, ReLU), and the ReLU's
    positive homogeneity + attn/r compensation keeps everything exact.
  - LN variance stats come from out-free-1 matmuls (sa = aggT @ 1,
    saa = aggT^2 @ 1) plus host-precomputed sums of x/u; softmax
    max-subtraction is replaced by a constant -100 shift (safe here).
  - PSUM tiles hold 2 windows each (8 banks -> 4 windows in flight).
"""

import sys

if "/opt/trn_rl_repo" not in sys.path:
    sys.path.insert(0, "/opt/trn_rl_repo")

import numpy as np

N, E, V_IN, HID, U_IN, B, MEM = 50000, 800000, 128, 128, 64, 64, 128
CAT = V_IN + HID + U_IN  # 320
ALPHA = 0.5
EPS = 1e-5
NCORES = 8
P = 128
N_LOC = N // NCORES        # 6250 nodes per core
NW = (N_LOC + P - 1) // P  # 49 windows of 128 nodes
N_PAD = NW * P             # 6272
G = 4                      # windows per DMA group
SPILL = 4                  # one-hot builds per window offloaded DVE -> Pool
OH_BUFS = 16
WK_BUFS = 6
ESPLIT = 4
AT_ON_ACT = False
AGG_ON_DVE = False
ATTN_ON_DVE = True
RS_ON_DVE = True
SQ_ON_ACT = False
SSUM_POOL = False
MU_ON_ACT = False
SPILL_ILV = False
EXP_SHIFT = -100.0         # constant softmax shift (replaces max-subtract)
DEBUG_TAP = None           # None|"sc"|"pt"|"attn"|"zT"|"hT": route tap to out


# ---------------------------------------------------------------------------
# Workarounds for this container's walrus: at most ONE sync wait per
# instruction is encodable. Tile's scheduler emits multi-waits; split them
# onto same-engine NoOps. Same for the TileContext exit drain.
# ---------------------------------------------------------------------------

def _patched_drain_and_barrier(self, tick_clock, wait_clock):
    from concourse.vector_clock import ScopedClock, VectorClock

    nc = self.nc
    gvc = tick_clock.global_clock
    nprocs = len(gvc)
    for proc in range(nprocs):
        tick = gvc[proc]
        if tick <= 0:
            continue
        one = VectorClock([0] * nprocs)
        one.require_at_least(proc, tick)
        inst = nc.sync.drain()
        wait_clock.add_sem_waits(inst.ins, ScopedClock({None: one}))
    nc.sync.drain()
    nc.all_engine_barrier()
    assert self.sems is not None
    popped = nc._tile_sem_poison_stack.pop()
    assert popped is self._sem_poison
    nc.clear_and_free_semaphores(list(self.sems.allocated().values()))
    nc.all_engine_barrier()


def _split_multi_waits(nc):
    from concourse import mybir

    for f in nc.m.functions:
        for bb in f.blocks:
            out = []
            for inst in bb.instructions:
                si = inst.sync_info
                if si is not None and si.on_wait is not None and len(si.on_wait) > 1:
                    waits = list(si.on_wait)
                    for i, w in enumerate(waits[:-1]):
                        out.append(mybir.InstNoOp(
                            name=f"{inst.name}-wsplit{i}",
                            engine=inst.engine,
                            sync_info=mybir.SyncInfo(on_wait=[w], on_update=[]),
                        ))
                    si.on_wait = waits[-1:]
                out.append(inst)
            bb.instructions[:] = out


_patch_applied = False


def _apply_patches():
    global _patch_applied
    if _patch_applied:
        return
    import concourse.tile as tile

    tile.TileContext._drain_and_barrier = _patched_drain_and_barrier
    _patch_applied = True


# ---------------------------------------------------------------------------
# Bass module builder. Kernel structure depends only on the per-window chunk
# counts C (shared across cores) and bias-zero flags, so cache on that.
# ---------------------------------------------------------------------------

_nc_cache = {}


def _window_offsets(C):
    """Column offsets of each window's edge block in d_ea and dstl in d_dl."""
    offs, doffs = [], []
    o = d = 0
    for w in range(NW):
        offs.append(o)
        doffs.append(d)
        o += C[w] * P
        d += C[w]
    return offs, o, doffs, d


def _build(key, split_waits=True):
    """key: (C, sb_zero, b2_zero); C = per-window 128-edge chunk counts."""
    import concourse.bass as bass
    import concourse.tile as tile
    from concourse import mybir

    C, sb_zero, b2_zero = key
    _apply_patches()
    f32 = mybir.dt.float32
    bf16 = mybir.dt.bfloat16
    woff, TOT, doff, DTOT = _window_offsets(C)
    # variable group sizes: singletons at the ends so the pipeline fills and
    # drains on short DMAs, big groups in the steady middle
    sizes = [G] * (NW // G)
    rem = NW - sum(sizes)
    if rem:
        sizes.append(rem)
    groups = []
    g0 = 0
    for s in sizes:
        groups.append((g0, g0 + s))
        g0 += s
    assert g0 == NW
    win2g = np.empty(NW, dtype=np.int64)
    wslot = np.empty(NW, dtype=np.int64)
    for gi, (a, b) in enumerate(groups):
        win2g[a:b] = gi
        wslot[a:b] = np.arange(b - a)

    nc = bass.Bass()
    d_ea = nc.dram_tensor("ea", [P, TOT], bf16, kind="ExternalInput")
    d_dl = nc.dram_tensor("dl", [P, DTOT], f32, kind="ExternalInput")
    d_x = nc.dram_tensor("x", [P, NW * P], bf16, kind="ExternalInput")
    d_ub = nc.dram_tensor("ub", [U_IN, NW * P], bf16, kind="ExternalInput")
    d_st = nc.dram_tensor("st", [P, NW * 3], f32, kind="ExternalInput")
    d_mw = nc.dram_tensor("mw", [CAT, 2 * P], bf16, kind="ExternalInput")
    d_mv1 = nc.dram_tensor("mv1", [MEM, HID], bf16, kind="ExternalInput")
    d_w2 = nc.dram_tensor("w2", [HID, HID], bf16, kind="ExternalInput")
    d_iota = nc.dram_tensor("iota", [P, P], bf16, kind="ExternalInput")
    d_idb = nc.dram_tensor("idb", [P, P], bf16, kind="ExternalInput")
    d_sb = nc.dram_tensor("sb", [1, MEM], bf16, kind="ExternalInput")
    d_out = nc.dram_tensor("out", [P, NW * HID], bf16, kind="ExternalOutput")

    KCH = [(0, 0, 128), (1, 128, 128), (2, 256, 64)]  # (j, cat offset, K)

    with tile.TileContext(nc) as tc:
        with (
            tc.tile_pool(name="const", bufs=1) as cpool,
            tc.tile_pool(name="eg", bufs=3) as egpool,
            tc.tile_pool(name="xg", bufs=3) as xgpool,
            tc.tile_pool(name="ug", bufs=3) as ugpool,
            tc.tile_pool(name="og", bufs=2) as ogpool,
            tc.tile_pool(name="oh", bufs=OH_BUFS) as ohpool,
            tc.tile_pool(name="asb", bufs=5) as asbpool,
            tc.tile_pool(name="sq", bufs=4) as sqpool,
            tc.tile_pool(name="small", bufs=12) as spool,
            tc.tile_pool(name="work", bufs=WK_BUFS) as wpool,
            tc.tile_pool(name="agg_ps", bufs=2, space="PSUM") as aggps,
            tc.tile_pool(name="scy_ps", bufs=3, space="PSUM") as scyps,
            tc.tile_pool(name="ht_ps", bufs=2, space="PSUM") as htps,
            tc.tile_pool(name="tr_ps", bufs=1, space="PSUM") as trps,
        ):
            # ---- constants ----
            # iota first: it gates the very first one-hot; the rest of the
            # constants are only needed ~5 stages later, so they are DMA'd
            # after group 0/1 edge loads (see emit_late_consts below)
            t_iota = cpool.tile([P, P], bf16)
            nc.scalar.dma_start(out=t_iota[:], in_=d_iota[:])
            t_mw = cpool.tile([P, 3, 2 * P], bf16)
            t_mv1 = cpool.tile([P, P], bf16)
            t_w2 = cpool.tile([P, P], bf16)
            t_idb = cpool.tile([P, P], bf16)
            t_st = cpool.tile([P, NW, 3], f32)
            if not sb_zero:
                t_sb = cpool.tile([1, MEM], bf16)

            def emit_late_consts():
                for j, off, K in KCH:
                    nc.sync.dma_start(out=t_mw[:K, j, :],
                                      in_=d_mw[off:off + K, :])
                nc.sync.dma_start(out=t_mv1[:], in_=d_mv1[:])
                nc.sync.dma_start(out=t_w2[:], in_=d_w2[:])
                nc.sync.dma_start(out=t_idb[:], in_=d_idb[:])
                nc.sync.dma_start(out=t_st[:], in_=d_st[:])
                if not sb_zero:
                    nc.sync.dma_start(out=t_sb[:1], in_=d_sb[:])
            t_ones_b = cpool.tile([P, 1], bf16)
            nc.vector.memset(t_ones_b[:], 1.0)
            t_one_row = cpool.tile([1, P], bf16)
            nc.vector.memset(t_one_row[:1], 1.0)
            t_shift = cpool.tile([P, 1], f32)
            nc.vector.memset(t_shift[:], EXP_SHIFT)
            t_zero = cpool.tile([P, 1], f32)
            nc.vector.memset(t_zero[:], 0.0)

            gtiles = {}     # gi -> (eg, dl, xg, ug, og, c0, d0)
            # per-stage 2-window PSUM tiles, keyed by w//2; each kind is
            # allocated on first touch by its own stage so at most its pool's
            # bufs halves are alive (8 banks total).
            aggd, scd, htd, trd, yd = {}, {}, {}, {}, {}
            s1d, s2d, a1d, hTd = {}, {}, {}, {}   # per-window scratch

            def load_group(gi):
                g0, g1 = groups[gi]
                gw = g1 - g0
                c0, c1 = woff[g0], woff[g1 - 1] + C[g1 - 1] * P
                d0, d1 = doff[g0], doff[g1 - 1] + C[g1 - 1]
                # small tensors first: dl gates the DVE one-hot stream, so it
                # must not queue behind the multi-us edge transfer
                dl = egpool.tile([P, d1 - d0], f32, tag="dl")
                nc.sync.dma_start(out=dl[:], in_=d_dl[:, d0:d1])
                xg = xgpool.tile([P, gw * P], bf16, tag="xg")
                nc.sync.dma_start(out=xg[:], in_=d_x[:, g0 * P:g1 * P])
                ug = ugpool.tile([U_IN, gw * P], bf16, tag="ug")
                nc.sync.dma_start(out=ug[:], in_=d_ub[:, g0 * P:g1 * P])
                eg = egpool.tile([P, c1 - c0], bf16, tag="eg")
                # split the bulk edge payload so the first windows of the
                # group become available sooner; for the first groups spread
                # the issue latency across queues so transfers start sooner
                bounds = [woff[g0 + i * gw // ESPLIT] for i in range(ESPLIT)
                          if i * gw // ESPLIT < gw] + [c1]
                bounds = sorted(set(bounds))
                qs = ([nc.sync] * 4 if gi < 2
                      else [nc.sync] * 4)
                for i, (a, b) in enumerate(zip(bounds, bounds[1:])):
                    qs[i % 4].dma_start(out=eg[:, a - c0:b - c0],
                                        in_=d_ea[:, a:b])
                og = ogpool.tile([P, gw * P], bf16, tag="og")
                gtiles[gi] = (eg, dl, xg, ug, og, c0, d0)

            def half(d, w, mk):
                h = w // 2
                if h not in d:
                    d[h] = mk()
                return d[h], w % 2

            def emit_seg(w):
                gi = int(win2g[w])
                eg, dl = gtiles[gi][0], gtiles[gi][1]
                c0, d0 = gtiles[gi][5], gtiles[gi][6]
                Cw = C[w]
                base = woff[w] - c0
                dbase = doff[w] - d0
                agg2, s2 = half(aggd, w,
                                lambda: aggps.tile([P, 2, 130], f32,
                                                   tag="agg2", name="agg2"))
                pagg = agg2[:, s2, 0:HID]
                for c in range(Cw):
                    oh = ohpool.tile([P, P], bf16, tag="oh")
                    spill_this = (c % 4 == 3) if SPILL_ILV else (c >= Cw - SPILL)
                    eng = nc.gpsimd if spill_this else nc.vector
                    eng.tensor_scalar(
                        out=oh[:], in0=t_iota[:],
                        scalar1=dl[:, dbase + c:dbase + c + 1], scalar2=None,
                        op0=mybir.AluOpType.is_equal,
                    )
                    nc.tensor.matmul(
                        pagg, lhsT=eg[:, base + c * P:base + (c + 1) * P],
                        rhs=oh[:], start=(c == 0), stop=(c == Cw - 1),
                        skip_group_check=True)

            def emit_stats1(w):
                agg2, s2 = half(aggd, w, None)
                pagg = agg2[:, s2, 0:HID]

                # aggT -> SBUF bf16 (Act), square it (Pool; bf16 is plenty
                # for the variance stat)
                aggT = asbpool.tile([P, P], bf16)
                if AGG_ON_DVE:
                    nc.vector.tensor_copy(out=aggT[:], in_=pagg)
                else:
                    nc.scalar.copy(out=aggT[:], in_=pagg)
                sq = sqpool.tile([P, P], bf16)
                if SQ_ON_ACT:
                    nc.scalar.activation(out=sq[:], in_=aggT[:],
                                         func=mybir.ActivationFunctionType.Square)
                else:
                    nc.gpsimd.tensor_tensor(out=sq[:], in0=aggT[:],
                                            in1=aggT[:],
                                            op=mybir.AluOpType.mult)

                # LN stats: sa = sum_h agg, saa = sum_h agg^2 (out-free-1 mms)
                nc.tensor.matmul(agg2[:, s2, 128:129], lhsT=aggT[:],
                                 rhs=t_ones_b[:], start=True, stop=True,
                                 skip_group_check=True)
                nc.tensor.matmul(agg2[:, s2, 129:130], lhsT=sq[:],
                                 rhs=t_ones_b[:], start=True, stop=True,
                                 skip_group_check=True)

                mu = spool.tile([P, 1], f32, tag="mu")
                if MU_ON_ACT:
                    nc.scalar.activation(
                        out=mu[:], in_=agg2[:, s2, 128:129],
                        func=mybir.ActivationFunctionType.Identity,
                        bias=t_st[:, w, 2:3], scale=1.0 / CAT)
                else:
                    nc.vector.tensor_scalar(
                        out=mu[:], in0=agg2[:, s2, 128:129],
                        scalar1=t_st[:, w, 0:1], scalar2=1.0 / CAT,
                        op0=mybir.AluOpType.add, op1=mybir.AluOpType.mult)
                musq = spool.tile([P, 1], f32, tag="musq")
                nc.gpsimd.tensor_tensor(out=musq[:], in0=mu[:], in1=mu[:],
                                        op=mybir.AluOpType.mult)
                bias_v = spool.tile([P, 1], f32, tag="bv")
                nc.gpsimd.tensor_scalar(
                    out=bias_v[:], in0=t_st[:, w, 1:2], scalar1=musq[:, :1],
                    scalar2=None, op0=mybir.AluOpType.subtract)
                s1d[w] = (aggT, bias_v)

            def emit_stats2(w):
                gi, s = int(win2g[w]), int(wslot[w])
                _, _, xg, ug, _, _, _ = gtiles[gi]
                ns = slice(s * P, (s + 1) * P)
                agg2, s2 = half(aggd, w, None)
                aggT, bias_v = s1d.pop(w)

                std = spool.tile([P, 1], f32, tag="std")
                nc.scalar.activation(out=std[:], in_=agg2[:, s2, 129:130],
                                     func=mybir.ActivationFunctionType.Sqrt,
                                     bias=bias_v[:, :1], scale=1.0 / CAT)
                r = spool.tile([P, 1], f32, tag="r")
                nc.vector.reciprocal(out=r[:], in_=std[:])

                # scores_raw[node, mem] = catT' @ centered MkgT
                sc2, _ = half(scd, w,
                              lambda: scyps.tile([P, 2, P], f32,
                                                 tag="scy", name="sc2"))
                psc = sc2[:, s2, :]
                nc.tensor.matmul(psc, lhsT=xg[:, ns], rhs=t_mw[:, 0, 0:P],
                                 start=True, stop=False, skip_group_check=True)
                nc.tensor.matmul(psc, lhsT=aggT[:], rhs=t_mw[:, 1, 0:P],
                                 start=False, stop=False, skip_group_check=True)
                nc.tensor.matmul(psc, lhsT=ug[:U_IN, ns],
                                 rhs=t_mw[:U_IN, 2, 0:P],
                                 start=False, stop=sb_zero,
                                 skip_group_check=True)
                if not sb_zero:
                    nc.tensor.matmul(psc, lhsT=t_one_row[:1, :],
                                     rhs=t_sb[:1, :], start=False, stop=True,
                                     skip_group_check=True)

                s2d[w] = (std, r, aggT)

            def emit_attn1(w):
                std, r, aggT = s2d[w]
                sc2, s2 = half(scd, w, None)

                # softmax over MEM without max-reduce: exp(r*raw + SHIFT)
                pt = wpool.tile([P, MEM], bf16, tag="pt")
                ssum = spool.tile([P, 1], f32, tag="ss")
                if SSUM_POOL:
                    nc.scalar.activation(out=pt[:], in_=sc2[:, s2, :],
                                         func=mybir.ActivationFunctionType.Exp,
                                         bias=t_shift[:, :1], scale=r[:, :1])
                    nc.gpsimd.tensor_reduce(out=ssum[:], in_=pt[:],
                                            axis=mybir.AxisListType.X,
                                            op=mybir.AluOpType.add)
                else:
                    nc.scalar.activation(out=pt[:], in_=sc2[:, s2, :],
                                         func=mybir.ActivationFunctionType.Exp,
                                         bias=t_shift[:, :1], scale=r[:, :1],
                                         accum_out=ssum[:, :1])
                issum = spool.tile([P, 1], f32, tag="is")
                nc.vector.reciprocal(out=issum[:], in_=ssum[:])
                # attn' = attn / r = pt * std / ssum
                rs = spool.tile([P, 1], f32, tag="rs")
                eng_rs = nc.vector if RS_ON_DVE else nc.gpsimd
                eng_rs.tensor_scalar(out=rs[:], in0=std[:],
                                     scalar1=issum[:, :1], scalar2=None,
                                     op0=mybir.AluOpType.mult)
                attn = wpool.tile([P, MEM], bf16, tag="at")
                eng_at = nc.vector if ATTN_ON_DVE else nc.gpsimd
                eng_at.tensor_scalar(out=attn[:], in0=pt[:],
                                     scalar1=rs[:, :1], scalar2=None,
                                     op0=mybir.AluOpType.mult)
                a1d[w] = attn

            def emit_attn2(w):
                gi, s = int(win2g[w]), int(wslot[w])
                _, _, xg, ug, _, _, _ = gtiles[gi]
                ns = slice(s * P, (s + 1) * P)
                _, _, aggT = s2d[w]
                attn = a1d.pop(w)

                # h_rawT[hid, node] = centered W1g^T @ catT' (transposed so
                # the ReLU output feeds h @ W2 with no extra transpose).
                # The whole ht2 accumulation group lives inside this one
                # stage so the two slots sharing the PSUM bank never have
                # interleaved open groups.
                ht2, s2 = half(htd, w,
                               lambda: htps.tile([P, 2, P], f32,
                                                 tag="ht2", name="ht2"))
                pht = ht2[:, s2, :]
                nc.tensor.matmul(pht, lhsT=t_mw[:, 0, P:2 * P],
                                 rhs=xg[:, ns],
                                 start=True, stop=False, skip_group_check=True)
                nc.tensor.matmul(pht, lhsT=t_mw[:, 1, P:2 * P], rhs=aggT[:],
                                 start=False, stop=False, skip_group_check=True)
                nc.tensor.matmul(pht, lhsT=t_mw[:U_IN, 2, P:2 * P],
                                 rhs=ug[:U_IN, ns],
                                 start=False, stop=False, skip_group_check=True)

                tr2, _ = half(trd, w,
                              lambda: trps.tile([P, 2, P], bf16,
                                                tag="tr2", name="tr2"))
                aT_ps = tr2[:, s2, :]
                nc.tensor.transpose(out=aT_ps, in_=attn[:], identity=t_idb[:])
                aT = wpool.tile([P, P], bf16, tag="aTs")
                if AT_ON_ACT:
                    nc.scalar.copy(out=aT[:], in_=aT_ps)
                else:
                    nc.vector.tensor_copy(out=aT[:], in_=aT_ps)
                # z2T[hid, node] += Mv1'^T-contract over mem: lhsT=mv1, rhs=aT
                nc.tensor.matmul(pht, lhsT=t_mv1[:], rhs=aT[:],
                                 start=False, stop=True, skip_group_check=True)

            def emit_relu(w):
                ht2, s2 = half(htd, w, None)
                # hT' = relu(zT); true h = r * hT' (applied at the y copy)
                hT = wpool.tile([P, HID], bf16, tag="hTs")
                nc.scalar.activation(out=hT[:], in_=ht2[:, s2, :],
                                     func=mybir.ActivationFunctionType.Relu,
                                     bias=t_zero[:, :1], scale=1.0)
                hTd[w] = hT

            def emit_tail(w):
                gi, s = int(win2g[w]), int(wslot[w])
                og = gtiles[gi][4]
                ns = slice(s * P, (s + 1) * P)
                std, r, _ = s2d.pop(w)
                hT = hTd.pop(w)

                # y = (r*hT')^T @ W2  -> scale by r during the PSUM->SBUF copy
                y2, s2 = half(yd, w,
                              lambda: scyps.tile([P, 2, HID], f32,
                                                 tag="scy", name="y2"))
                py = y2[:, s2, :]
                nc.tensor.matmul(py, lhsT=hT[:], rhs=t_w2[:],
                                 start=True, stop=True,
                                 skip_group_check=True)
                if DEBUG_TAP is None:
                    nc.scalar.activation(out=og[:, ns], in_=py,
                                         func=mybir.ActivationFunctionType.Copy,
                                         bias=0.0, scale=r[:, :1])

            gflushed = {}

            def flush_upto(w):
                # flush og columns up to window w (inclusive); issued from the
                # Act queue right after the y copies it depends on
                gi = int(win2g[w])
                g0, g1 = groups[gi]
                f0 = gflushed.get(gi, g0)
                og = gtiles[gi][4]
                nc.scalar.dma_start(out=d_out[:, f0 * P:(w + 1) * P],
                                    in_=og[:, (f0 - g0) * P:(w + 1 - g0) * P])
                gflushed[gi] = w + 1
                if w + 1 == g1:
                    del gtiles[gi]
                    gflushed.pop(gi, None)

            # 5-deep software pipeline: seg(w+4) | stats1(w+3) | stats2(w+2)
            # | attn(w+1) | tail(w). Every engine's per-iteration ops have
            # deps satisfied by previous iterations, so the in-order engine
            # streams never head-of-line block.
            # NOTE: within each iteration the short node-stage ops are emitted
            # BEFORE the bulky one-hot/seg block. The tile scheduler encodes
            # sync as per-engine tick waits, so putting the latency-critical
            # ops early in each engine's stream keeps their covering ticks
            # cheap for consumers on other engines.
            loaded = set()

            def ensure_loaded(gi):
                if gi < len(groups) and gi not in loaded:
                    loaded.add(gi)
                    load_group(gi)

            ensure_loaded(0)
            emit_late_consts()
            for w in range(-6, NW):
                wn = w + 6
                ensure_loaded(int(win2g[min(max(w + 6, 0), NW - 1)]))
                ensure_loaded(int(win2g[min(max(w + 10, 0), NW - 1)]))
                if wn < NW:
                    emit_seg(wn)
                if 0 <= w + 5 < NW:
                    emit_stats1(w + 5)
                if 0 <= w + 4 < NW:
                    emit_stats2(w + 4)
                if 0 <= w + 3 < NW:
                    emit_attn1(w + 3)
                if 0 <= w + 2 < NW:
                    emit_attn2(w + 2)
                if 0 <= w + 1 < NW:
                    emit_relu(w + 1)
                if w >= 0:
                    emit_tail(w)
                    gi_w = int(win2g[w])
                    if w == groups[gi_w][1] - 1:
                        flush_upto(w)
                    if w % 2 == 1:
                        aggd.pop(w // 2 - 1, None)
                        scd.pop(w // 2, None)
                        htd.pop(w // 2, None)
                        trd.pop(w // 2, None)
                        yd.pop(w // 2, None)

    if split_waits:
        _split_multi_waits(nc)
    return nc


def _prepare(x, edge_index, edge_attr, u, batch, Mk, Mv, ln_gamma, ln_beta,
             W1, b1, W2, b2):
    """Host-side sharding / packing. Returns (key, in_maps)."""
    import ml_dtypes
    bf = ml_dtypes.bfloat16

    x = np.asarray(x, dtype=np.float32)
    edge_attr = np.asarray(edge_attr, dtype=np.float32)
    u = np.asarray(u, dtype=np.float32)
    Mk = np.asarray(Mk, dtype=np.float32)
    Mv = np.asarray(Mv, dtype=np.float32)
    g = np.asarray(ln_gamma, dtype=np.float32)
    be = np.asarray(ln_beta, dtype=np.float32)
    W1 = np.asarray(W1, dtype=np.float32)
    b1 = np.asarray(b1, dtype=np.float32)
    W2 = np.asarray(W2, dtype=np.float32)
    b2 = np.asarray(b2, dtype=np.float32)
    dst = np.asarray(edge_index)[1].astype(np.int64)
    batch = np.asarray(batch).astype(np.int64)

    core_id = dst // N_LOC
    rem = dst - core_id * N_LOC
    w_id = rem >> 7
    loc = (rem & 127).astype(np.float32)
    skey = core_id * NW + w_id
    order = np.argsort(skey, kind="stable")
    counts = np.bincount(skey, minlength=NCORES * NW).reshape(NCORES, NW)
    # rank-align: each core processes its windows in descending-edge-count
    # order so the (shared) per-slot padded chunk count is the max of
    # same-rank counts across cores instead of same-physical-window counts
    perm = np.argsort(counts, axis=1, kind="stable")        # [NC, NW]
    # pyramid schedule: smallest windows at the START (fast pipeline fill)
    # and at the END (fast drain); largest in the middle where the pipeline
    # is saturated. Same rank pattern for all cores keeps C shared.
    asc = np.arange(NW)
    pyr = np.empty(NW, dtype=np.int64)
    pyr[0::2] = asc[:(NW + 1) // 2]          # even slots: small ranks
    pyr[1::2] = asc[(NW + 1) // 2:][::-1]    # odd slots: large ranks desc
    order_slots = np.argsort(np.concatenate([
        pyr[: NW // 3], pyr[NW // 3:][::-1]]), kind="stable")
    # simpler: put ranks in a ramp-up/ramp-down profile
    prof = np.empty(NW, dtype=np.int64)
    half1 = NW // 4
    prof[:half1] = asc[:half1]                        # small first
    prof[NW - half1:] = asc[half1:2 * half1][::-1]    # small-ish last
    prof[half1:NW - half1] = asc[2 * half1:]          # big in the middle
    perm = np.take_along_axis(perm, np.broadcast_to(prof, (NCORES, NW)), axis=1)
    rk_counts = np.take_along_axis(counts, perm, axis=1)
    C = np.maximum((rk_counts.max(axis=0) + P - 1) // P, 1).astype(np.int64)
    woff, TOT, doff, DTOT = _window_offsets([int(c) for c in C])

    starts = np.concatenate([[0], np.cumsum(counts.reshape(-1))])
    loc_sorted = loc[order]

    ea_pad = np.zeros((NCORES, P, TOT), dtype=bf)
    dl_pad = np.full((NCORES, P, DTOT), -1.0, dtype=np.float32)
    for c in range(NCORES):
        for v in range(NW):
            w = int(perm[c, v])          # physical window in virtual slot v
            k = c * NW + w
            s, e = starts[k], starts[k + 1]
            cnt = e - s
            Cw = int(C[v])
            base = woff[v]
            blkf = np.zeros((Cw * P, HID), dtype=np.float32)
            blkf[:cnt] = edge_attr[order[s:e]]
            ea_pad[c, :, base:base + Cw * P] = (
                blkf.astype(bf).reshape(Cw, P, HID)
                .transpose(1, 0, 2).reshape(P, Cw * P))
            lb = np.full(Cw * P, -1.0, dtype=np.float32)
            lb[:cnt] = loc_sorted[s:e]
            dl_pad[c, :, doff[v]:doff[v] + Cw] = lb.reshape(Cw, P).T

    u_b = u[batch]
    x_pad = np.zeros((NCORES, N_PAD, V_IN), dtype=np.float32)
    ub_pad = np.zeros((NCORES, N_PAD, U_IN), dtype=np.float32)
    x_pad[:, :N_LOC] = x.reshape(NCORES, N_LOC, V_IN)
    ub_pad[:, :N_LOC] = u_b.reshape(NCORES, N_LOC, U_IN)

    # transposed, window-blocked feature streams: [feat, v*128 + node],
    # window-permuted per core (virtual slot v = perm[c, v])
    ar = np.arange(NCORES)[:, None]
    xT = (x_pad.reshape(NCORES, NW, P, V_IN)[ar, perm]
          .transpose(0, 3, 1, 2).reshape(NCORES, V_IN, NW * P).astype(bf))
    ubT = (ub_pad.reshape(NCORES, NW, P, U_IN)[ar, perm]
           .transpose(0, 3, 1, 2).reshape(NCORES, U_IN, NW * P).astype(bf))

    # host-side LN stat constants: raw sum(x)+sum(u), and (sumsq)/CAT + eps
    sxu = x_pad.sum(axis=2) + ub_pad.sum(axis=2)                  # [NC, N_PAD]
    sq = ((x_pad ** 2).sum(axis=2) + (ub_pad ** 2).sum(axis=2)) / CAT + EPS
    st = np.stack([sxu, sq, sxu / CAT], axis=2)               # [NC, N_PAD, 3]
    st = (st.reshape(NCORES, NW, P, 3)[ar, perm].transpose(0, 2, 1, 3)
          .reshape(NCORES, P, NW * 3).astype(np.float32))

    mkgt = (Mk * g[None, :]).T                                    # [CAT, MEM]
    w1g = (1.0 - ALPHA) * g[:, None] * W1                         # [CAT, HID]
    mw = np.concatenate([mkgt, w1g], axis=1)                      # [CAT, 2P]
    # fold LN centering into the weights: sum_c (cat_c - mu) W_c
    #   == sum_c cat_c (W_c - mean_c W)   (exact)
    mw = mw - mw.mean(axis=0, keepdims=True)
    mw = np.ascontiguousarray(mw).astype(bf)
    sb = (Mk @ be).reshape(1, MEM)
    b1p = (1.0 - ALPHA) * (be @ W1) + b1
    mv1 = np.ascontiguousarray(ALPHA * (Mv @ W1) + b1p[None, :]).astype(bf)
    b2r = b2.reshape(1, HID)
    iota = np.tile(np.arange(P, dtype=np.float32), (P, 1)).astype(bf)
    ident = np.eye(P, dtype=np.float32).astype(bf)
    key = (tuple(int(v) for v in C),
           bool(np.all(sb == 0.0)), bool(np.all(b2r == 0.0)))
    _prepare.last_perm = perm

    in_maps = []
    for c in range(NCORES):
        in_maps.append({
            "ea": ea_pad[c], "dl": dl_pad[c],
            "x": xT[c], "ub": ubT[c], "st": st[c],
            "mw": mw, "mv1": mv1, "w2": W2.astype(bf),
            "sb": sb.astype(bf),
            "iota": iota, "idb": ident,
        })
    return key, in_maps


def kernel(**inputs):
    from concourse import bass_utils

    key, in_maps = _prepare(**inputs)
    nc = _nc_cache.get(key)
    if nc is None:
        nc = _build(key)
        _nc_cache[key] = nc
    res = bass_utils.run_bass_kernel_spmd(nc, in_maps, core_ids=list(range(NCORES)))
    perm = _prepare.last_perm
    outs = []
    for c, r in enumerate(res.results):
        o = r["out"].reshape(P, NW, HID).transpose(1, 0, 2)   # [v, node, HID]
        inv = np.empty(NW, dtype=np.int64)
        inv[perm[c]] = np.arange(NW)
        o = o[inv].reshape(N_PAD, HID)
        outs.append(o[:N_LOC])
    out = np.concatenate(outs, axis=0).astype(np.float32)
    # b2 is added host-side (constant vector; keeps the device epilogue to a
    # single scaled copy)
    return out + np.asarray(inputs["b2"], dtype=np.float32)[None, :]
